# revision 7
# baseline (speedup 1.0000x reference)
"""Distributed Bass attention kernel for trn2 (8 NeuronCores), v5.

Problem: B=4,H=16,T=2048,D=128 attention w/ Q/K/V linear projections.
  qp = q@Wq.T+bq ; kp = k@Wk.T+bk ; vp = v@Wv.T+bv
  S = qp@kp.T/sqrt(128); S = where(mask==1, -1e-9, S); P=softmax(S); out = P@vp

Key identities (v5):
  - masked logit -1e-9 ~= 0  =>  masked P_unnorm = exp(0) = 1; with the
    global shift C=ln(8): P = exp(S*scale - C), masked P = 0.125 exactly.
  - BLEND: P = (U - 0.125)*w + 0.125, U = exp(scale*S - C) (mask-
    oblivious), w = 1-m.  On device only (U-0.125)*w is computed:
      TS in-place (4x mode, [128,4096]): scr -= 0.125
      TT (2x mode, [128,4096]):          pt = scr * w
    The +0.125 constant is added ON THE HOST to the raw AV output:
    raw[...,e] += 0.125*colsum(vp)[e], raw[...,l] += 0.125*T (exact).
  - exp runs directly on the raw-logit PSUM (ScalarE does ONLY exp).
  - proj bias: DVE tensor_scalar(add bias[128,1]) fused into the
    PSUM->SBUF cast; vpx casts on ScalarE (copy); AV drain on DVE.
  - bv dropped on device: out = (P@vp)/l + bv applied on host (exact).
  - mask stored ic-major [128, (ic jt i)] so wide [128,4096] slices are
    contiguous (DVE fast modes need stride-1 2-byte operands).

Sharding: 64 (b,h) slabs -> 8 per core (head/data parallel, no collectives).

Per-core engine budget (measured rates): ScalarE exp 128x1970=252 +
vpx copies 18 = 270us; DVE TS 64x1127=72 + TT 64x2194=140 + proj casts
42 + drains 33 = 287us; PE S(110)+AV(121)+proj(14)+vproj(10) = 255us.
"""

import numpy as np
import ml_dtypes

import sys
sys.path.insert(0, "/opt/trn_rl_repo")

from concourse import bacc, bass, mybir
from concourse.tile import TileContext
from concourse.bass_utils import run_bass_kernel_spmd

B, H, T, D = 4, 16, 2048, 128
NCORES = 8
SPC = (B * H) // NCORES  # 8 slabs per core
NT = T // 128  # 16 j-tiles
IC = 1024  # i-chunk size
NCI = T // IC  # 2
SCALE = 1.0 / np.sqrt(D)
C_SHIFT = float(np.log(8.0))

F32 = mybir.dt.float32
BF16 = mybir.dt.bfloat16
AF = mybir.ActivationFunctionType
ALU = mybir.AluOpType


def _build_nc():
    nc = bacc.Bacc(target_bir_lowering=False, trn_type="TRN2")

    qt_d = nc.declare_dram_parameter("qt", [SPC * 128, T], BF16, isOutput=False)
    kt_d = nc.declare_dram_parameter("kt", [SPC * 128, T], BF16, isOutput=False)
    vt_d = nc.declare_dram_parameter("vt", [SPC * 128, T], BF16, isOutput=False)
    # wtb = (1-m) transposed, ic-major: [128, (ic jt i)]
    wtb_d = nc.declare_dram_parameter(
        "wtb", [128, NCI * NT * IC], BF16, isOutput=False
    )
    wqt_d = nc.declare_dram_parameter("wqt", [D, D], BF16, isOutput=False)
    wkt_d = nc.declare_dram_parameter("wkt", [D, D], BF16, isOutput=False)
    wvt_d = nc.declare_dram_parameter("wvt", [D, D], BF16, isOutput=False)
    bqc_d = nc.declare_dram_parameter("bqc", [D, 1], F32, isOutput=False)
    bkc_d = nc.declare_dram_parameter("bkc", [D, 1], F32, isOutput=False)
    # out blocks: row = (s*NCI + ic)*128 + p, col = t*129 + e, e==128 is l
    out_d = nc.declare_dram_parameter(
        "out", [SPC * NCI * 128, IC // 128 * 129], F32, isOutput=True
    )

    with TileContext(nc) as tc:
        with (
            tc.tile_pool(name="const", bufs=1) as const_pool,
            tc.tile_pool(name="mmt", bufs=1) as mmt_pool,
            tc.tile_pool(name="qkvt", bufs=2) as qkvt_pool,
            tc.tile_pool(name="proj", bufs=2) as proj_pool,
            tc.tile_pool(name="vpx", bufs=2) as vpx_pool,
            tc.tile_pool(name="scr", bufs=2) as scr_pool,
            tc.tile_pool(name="pt", bufs=2) as pt_pool,
            tc.tile_pool(name="fin", bufs=2) as fin_pool,
            tc.tile_pool(name="pj_ps", bufs=2, space="PSUM") as pjps_pool,
            tc.tile_pool(name="s_ps", bufs=1, space="PSUM") as sps_pool,
            tc.tile_pool(name="o_ps", bufs=2, space="PSUM") as ops_pool,
        ):
            # ---- constants; DMA order = first-use order (proj critical) ----
            wqt = const_pool.tile([128, 128], BF16, tag="wqt")
            nc.sync.dma_start(out=wqt[:, :], in_=wqt_d[:, :])
            wkt = const_pool.tile([128, 128], BF16, tag="wkt")
            nc.sync.dma_start(out=wkt[:, :], in_=wkt_d[:, :])

            # slab-0 q/k loads next so the proj->S pipeline starts asap
            qkv0 = [None, None, None]
            for idx, (name, srcd) in enumerate((("qT", qt_d), ("kT", kt_d))):
                t0 = qkvt_pool.tile([128, T], BF16, tag=name)
                nc.sync.dma_start(out=t0[:, :], in_=srcd[0:128, :])
                qkv0[idx] = t0

            bqc = const_pool.tile([128, 1], F32, tag="bqc")
            nc.sync.dma_start(out=bqc[:, :], in_=bqc_d[:, :])
            bkc = const_pool.tile([128, 1], F32, tag="bkc")
            nc.sync.dma_start(out=bkc[:, :], in_=bkc_d[:, :])

            # not-mask, ic-major layout; chunk ic=0 first, then ic=1
            wtb = mmt_pool.tile([128, NCI * NT * IC], BF16, tag="wtb")
            HC = NT * IC
            nc.sync.dma_start(out=wtb[:, 0:HC], in_=wtb_d[:, 0:HC])

            wvt = const_pool.tile([128, 128], BF16, tag="wvt")
            nc.sync.dma_start(out=wvt[:, :], in_=wvt_d[:, :])
            vT0 = qkvt_pool.tile([128, T], BF16, tag="vT")
            nc.sync.dma_start(out=vT0[:, :], in_=vt_d[0:128, :])
            qkv0[2] = vT0

            nc.sync.dma_start(out=wtb[:, HC : 2 * HC], in_=wtb_d[:, HC : 2 * HC])

            negc = const_pool.tile([128, 1], F32, tag="negc")
            nc.vector.memset(negc[:, :], -C_SHIFT)

            # ---- software-pipelined slab phases ----
            def load(s):
                if s == 0:
                    return qkv0
                tiles = []
                for name, src in (("qT", qt_d), ("kT", kt_d), ("vT", vt_d)):
                    t = qkvt_pool.tile([128, T], BF16, tag=name)
                    nc.sync.dma_start(
                        out=t[:, :], in_=src[s * 128 : (s + 1) * 128, :]
                    )
                    tiles.append(t)
                return tiles

            def proj(qT, kT):
                qpT = proj_pool.tile([128, T], BF16, tag="qpT")
                kpT = proj_pool.tile([128, T], BF16, tag="kpT")
                for c in range(T // 512):
                    for srcT, w, bc, dst in (
                        (qT, wqt, bqc, qpT),
                        (kT, wkt, bkc, kpT),
                    ):
                        pps = pjps_pool.tile([128, 512], F32, tag="pj")
                        nc.tensor.matmul(
                            pps[:, :],
                            w[:, :],
                            srcT[:, c * 512 : (c + 1) * 512],
                            start=True,
                            stop=True,
                        )
                        # bias-add fused into the PSUM->SBUF bf16 cast
                        nc.vector.tensor_scalar(
                            dst[:, c * 512 : (c + 1) * 512],
                            pps[:, :],
                            bc[:, :],
                            None,
                            ALU.add,
                        )
                return qpT, kpT

            def vproj(vT):
                # vpx: 16 blocks [128(t), 129] bf16; col 128 = 1.0 (for l)
                vpx = vpx_pool.tile([128, NT * 130], BF16, tag="vpx")
                nc.gpsimd.memset(vpx[:, :], 1.0)
                vpxv = vpx[:, :].rearrange("p (j n) -> p j n", j=NT)  # n=130
                for b4 in range(NT // 4):
                    vps = pjps_pool.tile([128, 512], F32, tag="pj")
                    for t4 in range(4):
                        nc.tensor.matmul(
                            vps[:, t4 * 128 : (t4 + 1) * 128],
                            vT[:, (b4 * 4 + t4) * 128 : (b4 * 4 + t4 + 1) * 128],
                            wvt[:, :],
                            start=(t4 == 0),
                            stop=(t4 == 3),
                        )
                    # PSUM->SBUF cast on ScalarE (keeps DVE free)
                    nc.scalar.copy(
                        vpxv[:, b4 * 4 : (b4 + 1) * 4, 0:128],
                        vps[:, :].rearrange("p (t n) -> p t n", t=4),
                    )
                return vpx, vpxv

            def sme(qpT, kpT, ic, pending_av=None):
                # S matmuls -> exp straight from PSUM -> wide fused blend.
                # AV groups of the previous chunk are emitted between quads.
                i0 = ic * IC
                pt = pt_pool.tile([128, NT * IC], BF16, tag="pt")
                for q in range(NT // 4):  # quad = 4 j-tiles
                    scr = scr_pool.tile([128, 4 * IC], BF16, tag="scr")
                    for pp in range(2):  # pairs within quad
                        st = sps_pool.tile([128, 2 * IC], F32, tag="s")
                        for o in range(2):
                            jt = 4 * q + 2 * pp + o
                            for h in range(IC // 512):
                                nc.tensor.matmul(
                                    st[
                                        :,
                                        o * IC + h * 512 : o * IC + (h + 1) * 512,
                                    ],
                                    kpT[:, jt * 128 : (jt + 1) * 128],
                                    qpT[:, i0 + h * 512 : i0 + (h + 1) * 512],
                                    start=True,
                                    stop=True,
                                )
                        nc.scalar.activation(
                            scr[:, pp * 2 * IC : (pp + 1) * 2 * IC],
                            st[:, :],
                            AF.Exp,
                            bias=negc[:, :],
                            scale=float(SCALE),
                        )
                        if pending_av is not None:
                            pending_av(2 * q + pp)
                    # scr -= 0.125 (in-place, 4x); pt = scr * w (2x)
                    nc.vector.tensor_scalar(
                        scr[:, :], scr[:, :], 0.125, None, ALU.subtract
                    )
                    nc.vector.tensor_tensor(
                        pt[:, 4 * q * IC : 4 * (q + 1) * IC],
                        scr[:, :],
                        wtb[:, ic * HC + 4 * q * IC : ic * HC + 4 * (q + 1) * IC],
                        ALU.mult,
                    )
                return pt

            def make_av(s, ic, pt, vpxv):
                ptv = pt[:, :].rearrange("p (j i) -> p j i", j=NT)
                ot8 = fin_pool.tile([128, IC // 128 * 129], F32, tag="ot8")

                def emit(itl):
                    io = itl * 129
                    ops = ops_pool.tile([128, 129], F32, tag="o")
                    for jt in range(NT):
                        nc.tensor.matmul(
                            ops[:, :],
                            ptv[:, jt, itl * 128 : itl * 128 + 128],
                            vpxv[:, jt, 0:129],
                            start=(jt == 0),
                            stop=(jt == NT - 1),
                        )
                    # raw O plus l (col 128); +0.125*colsum and the divide
                    # happen on the host
                    nc.vector.tensor_copy(ot8[:, io : io + 129], ops[:, :])
                    if itl == IC // 128 - 1:
                        r0 = (s * NCI + ic) * 128
                        nc.sync.dma_start(
                            out=out_d[r0 : r0 + 128, :], in_=ot8[:, :]
                        )

                return emit

            pending = None
            for s in range(SPC):
                qT, kT, vT = load(s)
                qpT, kpT = proj(qT, kT)
                vpx, vpxv = vproj(vT)
                for ic in range(NCI):
                    pt = sme(qpT, kpT, ic, pending)
                    pending = make_av(s, ic, pt, vpxv)
            for tp in range(NT // 2):  # flush last chunk's AV groups
                pending(tp)
    if not nc.is_finalized():
        nc.finalize()
    return nc


_NC_CACHE = None


def kernel(q, k, v, mask, Wq, bq, Wk, bk, Wv, bv):
    global _NC_CACHE
    if _NC_CACHE is None:
        _NC_CACHE = _build_nc()
    nc = _NC_CACHE

    bf16 = ml_dtypes.bfloat16

    # host-side layout transforms (per-core slab-major, transposed, bf16)
    qf = np.asarray(q, np.float32).reshape(B * H, T, D)
    kf = np.asarray(k, np.float32).reshape(B * H, T, D)
    vf = np.asarray(v, np.float32).reshape(B * H, T, D)
    qt = np.ascontiguousarray(qf.transpose(0, 2, 1)).astype(bf16)  # [64,128,T]
    kt = np.ascontiguousarray(kf.transpose(0, 2, 1)).astype(bf16)
    vt = np.ascontiguousarray(vf.transpose(0, 2, 1)).astype(bf16)
    # wtb[p, (ic jt i)] = 1 - m[jt*128+p, ic*IC+i]  (transposed not-mask)
    wt = 1.0 - np.asarray(mask, np.float32)[0, 0].T  # [j, i]
    wt = wt.reshape(NT, 128, NCI, IC).transpose(1, 2, 0, 3)  # [p, ic, jt, i]
    wtb = np.ascontiguousarray(wt.reshape(128, NCI * NT * IC)).astype(bf16)
    wqt = np.ascontiguousarray(np.asarray(Wq, np.float32).T).astype(bf16)
    wkt = np.ascontiguousarray(np.asarray(Wk, np.float32).T).astype(bf16)
    wvt = np.ascontiguousarray(np.asarray(Wv, np.float32).T).astype(bf16)
    bqc = np.asarray(bq, np.float32).reshape(D, 1).copy()
    bkc = np.asarray(bk, np.float32).reshape(D, 1).copy()
    bvf = np.asarray(bv, np.float32).reshape(1, 1, 1, D)

    # 0.125 * colsum(vp) per slab (exact, fp32) for the host-side re-add
    Wvf = np.asarray(Wv, np.float32)
    vpsum = vf.sum(axis=1) @ Wvf.T  # [64, 128]

    in_maps = []
    for c in range(NCORES):
        sl = slice(c * SPC, (c + 1) * SPC)
        in_maps.append(
            {
                "qt": np.ascontiguousarray(qt[sl].reshape(SPC * 128, T)),
                "kt": np.ascontiguousarray(kt[sl].reshape(SPC * 128, T)),
                "vt": np.ascontiguousarray(vt[sl].reshape(SPC * 128, T)),
                "wtb": wtb,
                "wqt": wqt,
                "wkt": wkt,
                "wvt": wvt,
                "bqc": bqc,
                "bkc": bkc,
            }
        )

    global _LAST_IN_MAPS
    _LAST_IN_MAPS = in_maps
    res = run_bass_kernel_spmd(nc, in_maps, core_ids=list(range(NCORES)))
    # out blocks: row=(s*NCI+ic)*128+p, col=t*129+e; col 128 of each block = l
    outs = [
        np.asarray(res.results[c]["out"]).reshape(SPC, NCI, 128, IC // 128, 129)
        for c in range(NCORES)
    ]
    raw = np.concatenate(outs, axis=0)  # [64, NCI, 128, 8, 129]
    # host-side +0.125*colsum(vpx): e-cols get 0.125*vpsum, l gets 0.125*T
    raw[..., :D] += 0.125 * vpsum[:, None, None, None, :]
    raw[..., D] += 0.125 * T
    full = raw[..., :D] / raw[..., D:]
    # i = ic*1024 + t*128 + p  ->  order (s, ic, t, p, e)
    full = full.transpose(0, 1, 3, 2, 4).reshape(B, H, T, D)
    return np.ascontiguousarray(full + bvf).astype(np.float32)


# revision 9
# speedup vs baseline: 1.3956x; 1.3956x over previous
"""Distributed Bass attention kernel for trn2 (8 NeuronCores), v5.

Problem: B=4,H=16,T=2048,D=128 attention w/ Q/K/V linear projections.
  qp = q@Wq.T+bq ; kp = k@Wk.T+bk ; vp = v@Wv.T+bv
  S = qp@kp.T/sqrt(128); S = where(mask==1, -1e-9, S); P=softmax(S); out = P@vp

Key identities (v5):
  - masked logit -1e-9 ~= 0  =>  masked P_unnorm = exp(0) = 1; with the
    global shift C=ln(8): P = exp(S*scale - C), masked P = 0.125 exactly.
  - BLEND: P = (U - 0.125)*w + 0.125, U = exp(scale*S - C) (mask-
    oblivious), w = 1-m.  On device only (U-0.125)*w is computed:
      TS in-place (4x mode, [128,4096]): scr -= 0.125
      TT (2x mode, [128,4096]):          pt = scr * w
    The +0.125 constant is added ON THE HOST to the raw AV output:
    raw[...,e] += 0.125*colsum(vp)[e], raw[...,l] += 0.125*T (exact).
  - exp runs directly on the raw-logit PSUM (ScalarE does ONLY exp).
  - proj bias: DVE tensor_scalar(add bias[128,1]) fused into the
    PSUM->SBUF cast; vpx casts on ScalarE (copy); AV drain on DVE.
  - bv dropped on device: out = (P@vp)/l + bv applied on host (exact).
  - mask stored ic-major [128, (ic jt i)] so wide [128,4096] slices are
    contiguous (DVE fast modes need stride-1 2-byte operands).

Sharding: 64 (b,h) slabs -> 8 per core (head/data parallel, no collectives).

Per-core engine budget (measured rates): ScalarE exp 128x1970=252 +
vpx copies 18 = 270us; DVE TS 64x1127=72 + TT 64x2194=140 + proj casts
42 + drains 33 = 287us; PE S(110)+AV(121)+proj(14)+vproj(10) = 255us.
"""

import numpy as np
import ml_dtypes

import sys
sys.path.insert(0, "/opt/trn_rl_repo")

from concourse import bacc, bass, mybir
from concourse.tile import TileContext
from concourse.bass_utils import run_bass_kernel_spmd

B, H, T, D = 4, 16, 2048, 128
NCORES = 8
SPC = (B * H) // NCORES  # 8 slabs per core
NT = T // 128  # 16 j-tiles
IC = 1024  # i-chunk size
NCI = T // IC  # 2
SCALE = 1.0 / np.sqrt(D)
C_SHIFT = float(np.log(8.0))

F32 = mybir.dt.float32
BF16 = mybir.dt.bfloat16
AF = mybir.ActivationFunctionType
ALU = mybir.AluOpType


def _build_nc():
    nc = bacc.Bacc(target_bir_lowering=False, trn_type="TRN2")

    qt_d = nc.declare_dram_parameter("qt", [SPC * 128, T], BF16, isOutput=False)
    kt_d = nc.declare_dram_parameter("kt", [SPC * 128, T], BF16, isOutput=False)
    vt_d = nc.declare_dram_parameter("vt", [SPC * 128, T], BF16, isOutput=False)
    # wtb = (1-m) transposed, ic-major: [128, (ic jt i)]
    wtb_d = nc.declare_dram_parameter(
        "wtb", [128, NCI * NT * IC], BF16, isOutput=False
    )
    wqt_d = nc.declare_dram_parameter("wqt", [D, D], BF16, isOutput=False)
    wkt_d = nc.declare_dram_parameter("wkt", [D, D], BF16, isOutput=False)
    wvt_d = nc.declare_dram_parameter("wvt", [D, D], BF16, isOutput=False)
    bqc_d = nc.declare_dram_parameter("bqc", [D, 1], F32, isOutput=False)
    bkc_d = nc.declare_dram_parameter("bkc", [D, 1], F32, isOutput=False)
    # out blocks: row = (s*NCI + ic)*128 + p, col = t*129 + e, e==128 is l
    out_d = nc.declare_dram_parameter(
        "out", [SPC * NCI * 128, IC // 128 * 129], F32, isOutput=True
    )

    with TileContext(nc) as tc:
        with (
            tc.tile_pool(name="const", bufs=1) as const_pool,
            tc.tile_pool(name="mmt", bufs=1) as mmt_pool,
            tc.tile_pool(name="qkvt", bufs=2) as qkvt_pool,
            tc.tile_pool(name="proj", bufs=2) as proj_pool,
            tc.tile_pool(name="vpx", bufs=2) as vpx_pool,
            tc.tile_pool(name="scr", bufs=2) as scr_pool,
            tc.tile_pool(name="pt", bufs=2) as pt_pool,
            tc.tile_pool(name="fin", bufs=2) as fin_pool,
            tc.tile_pool(name="pj_ps", bufs=2, space="PSUM") as pjps_pool,
            tc.tile_pool(name="s_ps", bufs=2, space="PSUM") as sps_pool,
            tc.tile_pool(name="o_ps", bufs=2, space="PSUM") as ops_pool,
        ):
            # ---- constants; DMA order = first-use order (proj critical) ----
            wqt = const_pool.tile([128, 128], BF16, tag="wqt")
            nc.sync.dma_start(out=wqt[:, :], in_=wqt_d[:, :])
            wkt = const_pool.tile([128, 128], BF16, tag="wkt")
            nc.sync.dma_start(out=wkt[:, :], in_=wkt_d[:, :])

            # slab-0 q/k loads next so the proj->S pipeline starts asap
            qkv0 = [None, None, None]
            for idx, (name, srcd) in enumerate((("qT", qt_d), ("kT", kt_d))):
                t0 = qkvt_pool.tile([128, T], BF16, tag=name)
                nc.sync.dma_start(out=t0[:, :], in_=srcd[0:128, :])
                qkv0[idx] = t0

            bqc = const_pool.tile([128, 1], F32, tag="bqc")
            nc.sync.dma_start(out=bqc[:, :], in_=bqc_d[:, :])
            bkc = const_pool.tile([128, 1], F32, tag="bkc")
            nc.sync.dma_start(out=bkc[:, :], in_=bkc_d[:, :])

            # not-mask, ic-major layout; chunk ic=0 first, then ic=1
            wtb = mmt_pool.tile([128, NCI * NT * IC], BF16, tag="wtb")
            HC = NT * IC
            nc.sync.dma_start(out=wtb[:, 0:HC], in_=wtb_d[:, 0:HC])

            wvt = const_pool.tile([128, 128], BF16, tag="wvt")
            nc.sync.dma_start(out=wvt[:, :], in_=wvt_d[:, :])
            vT0 = qkvt_pool.tile([128, T], BF16, tag="vT")
            nc.sync.dma_start(out=vT0[:, :], in_=vt_d[0:128, :])
            qkv0[2] = vT0

            nc.sync.dma_start(out=wtb[:, HC : 2 * HC], in_=wtb_d[:, HC : 2 * HC])

            negc = const_pool.tile([128, 1], F32, tag="negc")
            nc.vector.memset(negc[:, :], -C_SHIFT)

            # ---- software-pipelined slab phases ----
            def load(s):
                if s == 0:
                    return qkv0
                tiles = []
                for name, src in (("qT", qt_d), ("kT", kt_d), ("vT", vt_d)):
                    t = qkvt_pool.tile([128, T], BF16, tag=name)
                    nc.sync.dma_start(
                        out=t[:, :], in_=src[s * 128 : (s + 1) * 128, :]
                    )
                    tiles.append(t)
                return tiles

            def proj(qT, kT):
                qpT = proj_pool.tile([128, T], BF16, tag="qpT")
                kpT = proj_pool.tile([128, T], BF16, tag="kpT")
                for c in range(T // 512):
                    for srcT, w, bc, dst in (
                        (qT, wqt, bqc, qpT),
                        (kT, wkt, bkc, kpT),
                    ):
                        pps = pjps_pool.tile([128, 512], F32, tag="pj")
                        nc.tensor.matmul(
                            pps[:, :],
                            w[:, :],
                            srcT[:, c * 512 : (c + 1) * 512],
                            start=True,
                            stop=True,
                        )
                        # bias-add fused into the PSUM->SBUF bf16 cast
                        nc.vector.tensor_scalar(
                            dst[:, c * 512 : (c + 1) * 512],
                            pps[:, :],
                            bc[:, :],
                            None,
                            ALU.add,
                        )
                return qpT, kpT

            def vproj(vT):
                # vpx: 16 blocks [128(t), 129] bf16; col 128 = 1.0 (for l)
                vpx = vpx_pool.tile([128, NT * 130], BF16, tag="vpx")
                nc.gpsimd.memset(vpx[:, :], 1.0)
                vpxv = vpx[:, :].rearrange("p (j n) -> p j n", j=NT)  # n=130
                for b4 in range(NT // 4):
                    vps = pjps_pool.tile([128, 512], F32, tag="pj")
                    for t4 in range(4):
                        nc.tensor.matmul(
                            vps[:, t4 * 128 : (t4 + 1) * 128],
                            vT[:, (b4 * 4 + t4) * 128 : (b4 * 4 + t4 + 1) * 128],
                            wvt[:, :],
                            start=(t4 == 0),
                            stop=(t4 == 3),
                        )
                    # PSUM->SBUF cast on ScalarE (keeps DVE free)
                    nc.scalar.copy(
                        vpxv[:, b4 * 4 : (b4 + 1) * 4, 0:128],
                        vps[:, :].rearrange("p (t n) -> p t n", t=4),
                    )
                return vpx, vpxv

            def sme(qpT, kpT, ic, pending_av=None):
                # S matmuls -> exp straight from PSUM -> wide fused blend.
                # AV groups of the previous chunk are emitted between quads.
                i0 = ic * IC
                pt = pt_pool.tile([128, NT * IC], BF16, tag="pt")
                for q in range(NT // 4):  # quad = 4 j-tiles
                    scr = scr_pool.tile([128, 4 * IC], BF16, tag="scr")
                    for v4 in range(4):  # j-tiles within quad
                        jt = 4 * q + v4
                        st = sps_pool.tile([128, IC], F32, tag="s")
                        for h in range(IC // 512):
                            nc.tensor.matmul(
                                st[:, h * 512 : (h + 1) * 512],
                                kpT[:, jt * 128 : (jt + 1) * 128],
                                qpT[:, i0 + h * 512 : i0 + (h + 1) * 512],
                                start=True,
                                stop=True,
                            )
                        nc.scalar.activation(
                            scr[:, v4 * IC : (v4 + 1) * IC],
                            st[:, :],
                            AF.Exp,
                            bias=negc[:, :],
                            scale=float(SCALE),
                        )
                        if pending_av is not None and v4 % 2 == 1:
                            pending_av(2 * q + v4 // 2)
                    # scr -= 0.125 (in-place, 4x); pt = scr * w (2x)
                    nc.vector.tensor_scalar(
                        scr[:, :], scr[:, :], 0.125, None, ALU.subtract
                    )
                    nc.vector.tensor_tensor(
                        pt[:, 4 * q * IC : 4 * (q + 1) * IC],
                        scr[:, :],
                        wtb[:, ic * HC + 4 * q * IC : ic * HC + 4 * (q + 1) * IC],
                        ALU.mult,
                    )
                return pt

            def make_av(s, ic, pt, vpxv):
                ptv = pt[:, :].rearrange("p (j i) -> p j i", j=NT)
                ot8 = fin_pool.tile([128, IC // 128 * 129], F32, tag="ot8")

                def emit(itl):
                    io = itl * 129
                    ops = ops_pool.tile([128, 129], F32, tag="o")
                    for jt in range(NT):
                        nc.tensor.matmul(
                            ops[:, :],
                            ptv[:, jt, itl * 128 : itl * 128 + 128],
                            vpxv[:, jt, 0:129],
                            start=(jt == 0),
                            stop=(jt == NT - 1),
                        )
                    # raw O plus l (col 128); +0.125*colsum and the divide
                    # happen on the host
                    nc.vector.tensor_copy(ot8[:, io : io + 129], ops[:, :])
                    if itl == IC // 128 - 1:
                        r0 = (s * NCI + ic) * 128
                        nc.sync.dma_start(
                            out=out_d[r0 : r0 + 128, :], in_=ot8[:, :]
                        )

                return emit

            pending = None
            for s in range(SPC):
                qT, kT, vT = load(s)
                qpT, kpT = proj(qT, kT)
                vpx, vpxv = vproj(vT)
                for ic in range(NCI):
                    pt = sme(qpT, kpT, ic, pending)
                    pending = make_av(s, ic, pt, vpxv)
            for tp in range(NT // 2):  # flush last chunk's AV groups
                pending(tp)
    if not nc.is_finalized():
        nc.finalize()
    return nc


_NC_CACHE = None


def kernel(q, k, v, mask, Wq, bq, Wk, bk, Wv, bv):
    global _NC_CACHE
    if _NC_CACHE is None:
        _NC_CACHE = _build_nc()
    nc = _NC_CACHE

    bf16 = ml_dtypes.bfloat16

    # host-side layout transforms (per-core slab-major, transposed, bf16)
    qf = np.asarray(q, np.float32).reshape(B * H, T, D)
    kf = np.asarray(k, np.float32).reshape(B * H, T, D)
    vf = np.asarray(v, np.float32).reshape(B * H, T, D)
    qt = np.ascontiguousarray(qf.transpose(0, 2, 1)).astype(bf16)  # [64,128,T]
    kt = np.ascontiguousarray(kf.transpose(0, 2, 1)).astype(bf16)
    vt = np.ascontiguousarray(vf.transpose(0, 2, 1)).astype(bf16)
    # wtb[p, (ic jt i)] = 1 - m[jt*128+p, ic*IC+i]  (transposed not-mask)
    wt = 1.0 - np.asarray(mask, np.float32)[0, 0].T  # [j, i]
    wt = wt.reshape(NT, 128, NCI, IC).transpose(1, 2, 0, 3)  # [p, ic, jt, i]
    wtb = np.ascontiguousarray(wt.reshape(128, NCI * NT * IC)).astype(bf16)
    wqt = np.ascontiguousarray(np.asarray(Wq, np.float32).T).astype(bf16)
    wkt = np.ascontiguousarray(np.asarray(Wk, np.float32).T).astype(bf16)
    wvt = np.ascontiguousarray(np.asarray(Wv, np.float32).T).astype(bf16)
    bqc = np.asarray(bq, np.float32).reshape(D, 1).copy()
    bkc = np.asarray(bk, np.float32).reshape(D, 1).copy()
    bvf = np.asarray(bv, np.float32).reshape(1, 1, 1, D)

    # 0.125 * colsum(vp) per slab (exact, fp32) for the host-side re-add
    Wvf = np.asarray(Wv, np.float32)
    vpsum = vf.sum(axis=1) @ Wvf.T  # [64, 128]

    in_maps = []
    for c in range(NCORES):
        sl = slice(c * SPC, (c + 1) * SPC)
        in_maps.append(
            {
                "qt": np.ascontiguousarray(qt[sl].reshape(SPC * 128, T)),
                "kt": np.ascontiguousarray(kt[sl].reshape(SPC * 128, T)),
                "vt": np.ascontiguousarray(vt[sl].reshape(SPC * 128, T)),
                "wtb": wtb,
                "wqt": wqt,
                "wkt": wkt,
                "wvt": wvt,
                "bqc": bqc,
                "bkc": bkc,
            }
        )

    global _LAST_IN_MAPS
    _LAST_IN_MAPS = in_maps
    res = run_bass_kernel_spmd(nc, in_maps, core_ids=list(range(NCORES)))
    # out blocks: row=(s*NCI+ic)*128+p, col=t*129+e; col 128 of each block = l
    outs = [
        np.asarray(res.results[c]["out"]).reshape(SPC, NCI, 128, IC // 128, 129)
        for c in range(NCORES)
    ]
    raw = np.concatenate(outs, axis=0)  # [64, NCI, 128, 8, 129]
    # host-side +0.125*colsum(vpx): e-cols get 0.125*vpsum, l gets 0.125*T
    raw[..., :D] += 0.125 * vpsum[:, None, None, None, :]
    raw[..., D] += 0.125 * T
    full = raw[..., :D] / raw[..., D:]
    # i = ic*1024 + t*128 + p  ->  order (s, ic, t, p, e)
    full = full.transpose(0, 1, 3, 2, 4).reshape(B, H, T, D)
    return np.ascontiguousarray(full + bvf).astype(np.float32)
